# revision 26
# baseline (speedup 1.0000x reference)
"""Trainium2 Bass kernel for the dual-stream conv + cross-width attention module.

Sharding: 8 cores = (batch b, H-half). Core computes rows [h0, h0+96) of batch
b for both outputs (left/right packed along channels).

All conv matmuls are uniform (K=128, M=128, tile (0,0), N=388, fp16): mixing
tile_position / K=64 matmuls into the stream serializes every LDWEIGHTS
against the in-flight matmul and keeps the PE clock cold (measured 460ns/MM
vs 166ns/MM for a uniform stream). dy2 conv1 weights are zero-padded to
K=128; conv2/conv3 hold both fe streams in one tile (partitions = (s, ci))
with block-diagonal weights, and dy is handled by row-offset reads, so no
shifted copies are needed.

On-chip pipeline per 12-row block:
  conv1 (dy01-packed + zero-padded dy2, M=128 = fe1|fe2) -> relu -> c1c tiles
  conv2 (9 MMs: 3dy x 3dx block-diag) + skip-add         -> cs tiles
  conv3 (9 MMs)                                          -> Q1Q2 / K1K2 tiles
  per-row attention: 4 score matmuls (fp32r, N=256-padded), exp+rowsum on ACT,
  4 fused apply matmuls in transposed (w, c) layout, softmax normalization
  folded into per-partition scales. Host untransposes outputs.
"""

import sys

sys.path.insert(0, "/opt/trn_rl_repo")

from contextlib import ExitStack

import numpy as np
import ml_dtypes

import bass_rust
import concourse.bass as bass
import concourse.bacc as bacc
import concourse.mybir as mybir
from concourse import tile
from concourse.vector_clock import ScopedClock
from concourse import tile_utils

# ----------------------------------------------------------------------------
# Workaround: walrus in this container rejects Drain instructions with >1 sem
# wait ("Too many sync wait commands"). Split the TileContext tail drain into
# one-wait-per-Drain.
def _patched_drain_and_barrier(self, tick_clock, wait_clock):
    nc = self.nc
    drain_inst = nc.sync.drain()
    wait_clock.add_sem_waits(
        drain_inst.ins, ScopedClock({None: tick_clock.global_clock})
    )
    si = drain_inst.ins.sync_info
    if si is not None and len(si.on_wait) > 1:
        waits = list(si.on_wait)
        drain_inst.ins.sync_info = bass_rust.SyncInfo(
            on_wait=[waits[0]], on_update=list(si.on_update)
        )
        for w in waits[1:]:
            d = nc.sync.drain()
            d.ins.sync_info = bass_rust.SyncInfo(on_wait=[w], on_update=[])
    nc.all_engine_barrier()
    assert self.sems is not None
    popped = nc._tile_sem_poison_stack.pop()
    assert popped is self._sem_poison
    nc.clear_and_free_semaphores(list(self.sems.allocated().values()))
    nc.all_engine_barrier()


tile.TileContext._drain_and_barrier = _patched_drain_and_barrier
tile_utils.max_sbuf_usage = 206 * 1024

# ----------------------------------------------------------------------------
B, C, H, W = 4, 64, 192, 192
NCORES = 8
HLOC = 96            # rows per core
R = 12               # rows per block
NBLK = HLOC // R
WP = 194             # padded width
F32 = mybir.dt.float32
F32R = mybir.dt.float32r
F16 = mybir.dt.float16
BF16 = mybir.dt.bfloat16
AF = mybir.ActivationFunctionType
ALU = mybir.AluOpType
BF = ml_dtypes.bfloat16

# packed weight column layout (see _pack_weights)
C1A = 0                  # 3 x 128  conv1 dy01 (rows: dy0 ci | dy1 ci; M: fe1|fe2)
C1B = 384                # 3 x 128  conv1 dy2 (rows 0:64 zero, 64:128 dy2 ci)
C2A = [768, 768 + 384]   # per ws: 3 x 64 A (ws1 swapped: dy1|dy0)
C2B = [960, 960 + 384]   # per ws: 3 x 64 B dy2 (K=128 read; zero off-half)
C3A = [1536, 1536 + 384]  # conv3, normal + swapped variants
C3B = [1728, 1728 + 384]
IDC = 2304               # 128-col identity (skip-add via PE accumulation)
BCOL = 2432              # 2432 b1pack, 2433 b2pack, 2434 cbpack
WCOLS = 2435

IN_ROWS = R + 6          # in_dup lo rows   (base row r0-3)
C1_ROWS = R + 4          # c1 tile rows     (base r0-2)
CS_ROWS = R + 2          # cs tile rows     (base r0-1)
XS_ROWS = R + 2          # xskip rows       (base r0-1)


def _rr(ap, rows, lead=1):
    """view of a padded-row region as [P, rows, WP], starting at col `lead`."""
    return ap[:, lead:lead + rows * WP].rearrange("p (r c) -> p r c", c=WP)


def build_program(nblk=NBLK):
    nc = bacc.Bacc("TRN2", target_bir_lowering=False, debug=False,
                   num_devices=NCORES)

    x1d = nc.dram_tensor("x1pad", [64, 103 * WP], F16, kind="ExternalInput").ap()
    x2d = nc.dram_tensor("x2pad", [64, 103 * WP], F16, kind="ExternalInput").ap()
    xsd = nc.dram_tensor("xskip", [128, (HLOC + 2) * WP], F16, kind="ExternalInput").ap()
    ftd = nc.dram_tensor("fT", [192, HLOC * 128], BF16, kind="ExternalInput").ap()
    ffd = nc.dram_tensor("fF", [192, HLOC * 128], F32, kind="ExternalInput").ap()
    wpd = nc.dram_tensor("wpk", [128, WCOLS], F32, kind="ExternalInput").ap()
    emd = nc.dram_tensor("emask", [128, 6 * WP], F16, kind="ExternalInput").ap()
    outd = nc.dram_tensor("outT", [192, HLOC * 128], F32, kind="ExternalOutput").ap()

    with tile.TileContext(nc) as tc, ExitStack() as ctx:
        P = lambda **kw: ctx.enter_context(tc.tile_pool(**kw))
        wpool = P(name="w", bufs=1)
        iop = P(name="io", bufs=2)
        xskp = P(name="xsk", bufs=2)
        c1p = P(name="c1", bufs=1)
        csp = P(name="cs", bufs=1)
        qkp = P(name="qk", bufs=1)
        ftp = P(name="ft", bufs=2)
        lrp = P(name="lr", bufs=1)
        sep = P(name="se", bufs=3)
        smp = P(name="sm", bufs=3)
        pcv = P(name="pcv", bufs=2, space="PSUM")
        psc = P(name="psc", bufs=2, space="PSUM")
        pap_p = P(name="pap", bufs=2, space="PSUM")

        gsb_ring = [wpool.tile([128, 64], BF16, tag=f"gsb{i}", name=f"gsb{i}")
                    for i in range(3)]
        g2b_ring = [wpool.tile([128, 64], BF16, tag=f"g2b{i}", name=f"g2b{i}")
                    for i in range(3)]
        for t in gsb_ring + g2b_ring:
            nc.vector.memset(t[64:128, :], 0.0)
        wf = wpool.tile([128, WCOLS], F32)
        nc.sync.dma_start(wf[:], wpd[:])
        wb = wpool.tile([128, BCOL], F16)
        nc.vector.tensor_copy(wb[:], wf[:, 0:BCOL])
        emk = wpool.tile([128, 6 * WP], F16)
        nc.sync.dma_start(emk[:], emd[:])

        b1 = wf[:, BCOL:BCOL + 1]
        b2 = wf[:, BCOL + 1:BCOL + 2]
        cb = wf[:, BCOL + 2:BCOL + 3]

        copy_ctr = [0]

        def copy_bias(dst_ap, src_ap, bias_ap, relu):
            """dst = [relu](src + bias); alternate DVE/ACT for balance."""
            copy_ctr[0] += 1
            if copy_ctr[0] % 2 == 0:
                nc.scalar.activation(dst_ap, src_ap,
                                     AF.Relu if relu else AF.Identity,
                                     bias=bias_ap)
            elif relu:
                nc.vector.tensor_scalar(dst_ap, src_ap, bias_ap, 0.0,
                                        ALU.add, ALU.max)
            else:
                nc.vector.tensor_scalar(dst_ap, src_ap, bias_ap, None, ALU.add)

        def pad_zero(t, rows):
            nc.vector.memset(
                t[:, 0:rows * WP].rearrange("p (r c) -> p r c", c=WP)[:, :, 0:2],
                0.0)
            nc.vector.memset(t[:, rows * WP:rows * WP + 2], 0.0)

        def lo_slice(s):
            # stream parity s: 0 -> lo rows in partitions 0:64 (loHi),
            #                  1 -> lo rows in partitions 64:128 (hiLo)
            return slice(64, 128) if s else slice(0, 64)

        def hi_dma_inc(t, u, s):
            # after lo rows [u, u+2) are written, copy the shifted hi rows
            # they enable: hi[r] = lo[r+1] for r in [max(u-1,0), u+1).
            lo, hi = (slice(0, 64), slice(64, 128)) if s == 0 else \
                     (slice(64, 128), slice(0, 64))
            rlo = max(u - 1, 0)
            nc.sync.dma_start(t[hi, 1 + rlo * WP:1 + u * WP + WP],
                              t[lo, 1 + (rlo + 1) * WP:1 + (u + 2) * WP])

        def edge_mask(t, rows, s, u, mcol):
            # multiply lo rows [u, u+2) by edge mask rows [mcol, mcol+2)
            sl = lo_slice(s)
            nc.vector.tensor_mul(
                _rr(t, rows)[sl, u:u + 2, 1:193],
                _rr(t, rows)[sl, u:u + 2, 1:193],
                emk[sl, mcol * WP:(mcol + 2) * WP].rearrange(
                    "p (r c) -> p r c", c=WP)[:, :, 1:193])

        n = 2

        def emit_dmas(blk):
            r0 = blk * R
            in1 = iop.tile([128, 2 + IN_ROWS * WP], F16, tag="in1")
            in2 = iop.tile([128, 2 + IN_ROWS * WP], F16, tag="in2")
            for t, src in ((in1, x1d), (in2, x2d)):
                nc.sync.dma_start(t[0:64, 1:1 + IN_ROWS * WP],
                                  src[:, r0 * WP:(r0 + IN_ROWS) * WP])
                nc.sync.dma_start(t[64:128, 1:1 + (IN_ROWS - 1) * WP],
                                  t[0:64, 1 + WP:1 + IN_ROWS * WP])
            xs1 = xskp.tile([128, XS_ROWS * WP], F16, tag="xs1")
            xs2 = xskp.tile([128, XS_ROWS * WP], F16, tag="xs2")
            nc.sync.dma_start(xs1[0:64, :], xsd[0:64, r0 * WP:(r0 + XS_ROWS) * WP])
            nc.sync.dma_start(xs1[64:128, :], xsd[0:64, r0 * WP:(r0 + XS_ROWS) * WP])
            nc.sync.dma_start(xs2[0:64, :], xsd[64:128, r0 * WP:(r0 + XS_ROWS) * WP])
            nc.sync.dma_start(xs2[64:128, :], xsd[64:128, r0 * WP:(r0 + XS_ROWS) * WP])
            fta = ftp.tile([128, R * 128], BF16, tag="fta")
            ftb = ftp.tile([128, R * 128], BF16, tag="ftb")
            nc.sync.dma_start(fta[:], ftd[0:128, r0 * 128:(r0 + R) * 128])
            nc.sync.dma_start(ftb[0:64, :], ftd[128:192, r0 * 128:(r0 + R) * 128])
            # rows 64:128 must be zero: they are the K-padding of the w-hi
            # apply matmul chunks (finite lhsT junk x 0 = 0).
            nc.vector.memset(ftb[64:128, :], 0.0)
            ffa = ftp.tile([128, R * 128], F32, tag="ffa")
            ffb = ftp.tile([64, R * 128], F32, tag="ffb")
            nc.sync.dma_start(ffa[:], ffd[0:128, r0 * 128:(r0 + R) * 128])
            nc.sync.dma_start(ffb[:], ffd[128:192, r0 * 128:(r0 + R) * 128])
            return dict(in1=in1, in2=in2, xs1=xs1, xs2=xs2, fta=fta, ftb=ftb,
                        ffa=ffa, ffb=ffb)

        def conv1_jobs(blk, tl):
            # out rows: [r0-2, r0+R+2); per-stream hiLo tiles for dy01
            # packing. Returned as closures so a block's conv1 can interleave
            # into the previous block's attention steps (fills tensor stalls).
            r0 = blk * R
            c1t = [c1p.tile([128, 2 + C1_ROWS * WP], F16, tag=f"c1_{i}",
                            name=f"c1_{i}") for i in range(4)]
            jobs = []
            for ii0 in range(2):
                for y00 in range(r0 - 2, r0 + R + 2, 2):
                    def job(ii=ii0, y0=y00, r0=r0):
                        ind = (tl["in1"], tl["in2"])[ii]
                        ps = pcv.tile([128, 388], F32, tag="cv", name="cv")
                        for dx in range(3):
                            ca = 1 + (y0 - 1 - (r0 - 3)) * WP + dx - 1
                            chb = 1 + (y0 - (r0 - 3)) * WP + dx - 1
                            nc.tensor.matmul(
                                ps[:, 0:n * WP],
                                wb[:, C1A + dx * 128:C1A + dx * 128 + 128],
                                ind[:, ca:ca + n * WP], start=(dx == 0),
                                stop=False)
                            nc.tensor.matmul(
                                ps[:, 0:n * WP],
                                wb[:, C1B + dx * 128:C1B + dx * 128 + 128],
                                ind[:, chb:chb + n * WP], start=False,
                                stop=(dx == 2))
                        u = y0 - (r0 - 2)
                        for s in range(2):
                            sl = lo_slice(s)
                            psl = slice(s * 64, s * 64 + 64)
                            copy_bias(
                                _rr(c1t[2 * ii + s], C1_ROWS)[sl, u:u + n, 1:193],
                                _rr(ps, n, 0)[psl, :, 1:193],
                                b1[psl], relu=True)
                            hi_dma_inc(c1t[2 * ii + s], u, s)
                    jobs.append(job)

            def fixup():
                for i, t in enumerate(c1t):
                    s = i % 2
                    if blk == 0:
                        edge_mask(t, C1_ROWS, s, 0, 0)            # rows -2,-1
                        # hi[0] = lo[1] (abs -1): em rows (1,2) = (top, 1.0)
                        edge_mask(t, C1_ROWS, 1 - s, 0, 1)
                    if blk == NBLK - 1:
                        edge_mask(t, C1_ROWS, s, C1_ROWS - 2, 4)  # rows 96,97
                        edge_mask(t, C1_ROWS, 1 - s, C1_ROWS - 3, 4)
                    pad_zero(t, C1_ROWS)
            return c1t, jobs, fixup

        tl_cur = emit_dmas(0)
        c1t_cur, jobs0, fix0 = conv1_jobs(0, tl_cur)
        for j in jobs0:
            j()
        fix0()

        for blk in range(nblk):
            r0 = blk * R  # core-relative block start row
            c1t = c1t_cur
            xs1, xs2 = tl_cur["xs1"], tl_cur["xs2"]
            fta, ftb = tl_cur["fta"], tl_cur["ftb"]
            ffa, ffb = tl_cur["ffa"], tl_cur["ffb"]

            # ---------------- conv2 + skip ----------------
            # out rows: [r0-1, r0+R+1). Two K=128,M=64 chains per y0,
            # interleaved at col tiles (0,0)/(0,64) -> concurrent execution.
            cst = [csp.tile([128, 2 + CS_ROWS * WP], F16, tag=f"cs_{i}",
                            name=f"cs_{i}") for i in range(4)]
            for pair, xsk in ((0, xs1), (1, xs2)):
                for y0 in range(r0 - 1, r0 + R + 1, 2):
                    ps = pcv.tile([128, 388], F32, tag="cv")
                    for k in range(6):
                        for s in range(2):
                            srct = c1t[2 * pair + s]
                            po = s * 64
                            if k < 3:
                                dx = k
                                ca = 1 + (y0 - 1 - (r0 - 2)) * WP + dx - 1
                                nc.tensor.matmul(
                                    ps[po:po + 64, 0:n * WP],
                                    wb[:, C2A[s] + dx * 64:C2A[s] + dx * 64 + 64],
                                    srct[:, ca:ca + n * WP], start=(k == 0),
                                    stop=False, tile_position=(0, po))
                            else:
                                dx = k - 3
                                chb = 1 + (y0 - (r0 - 2)) * WP + dx - 1
                                nc.tensor.matmul(
                                    ps[po:po + 64, 0:n * WP],
                                    wb[:, C2B[s] + dx * 64:C2B[s] + dx * 64 + 64],
                                    srct[:, chb:chb + n * WP], start=False,
                                    stop=False, tile_position=(0, po))
                    u = y0 - (r0 - 1)
                    # skip-add folded into the accumulation (identity matmul)
                    nc.tensor.matmul(
                        ps[:, 0:n * WP], wb[:, IDC:IDC + 128],
                        xsk[:, u * WP:(u + n) * WP], start=False, stop=True)
                    for s in range(2):
                        sl = lo_slice(s)
                        psl = slice(s * 64, s * 64 + 64)
                        dst = _rr(cst[2 * pair + s], CS_ROWS)[sl, u:u + n, 1:193]
                        srcp = _rr(ps, n, 0)[psl, :, 1:193]
                        if s == 0:
                            nc.scalar.activation(dst, srcp, AF.Identity,
                                                 bias=b2[psl])
                        else:
                            nc.vector.tensor_scalar(dst, srcp, b2[psl],
                                                    None, ALU.add)
                        hi_dma_inc(cst[2 * pair + s], u, s)

            for i, t in enumerate(cst):
                s = i % 2
                if blk == 0:
                    edge_mask(t, CS_ROWS, s, 0, 1)            # rows -1,0
                    # hi[0] = lo[1] = abs row 0: unmasked; nothing to do
                if blk == NBLK - 1:
                    edge_mask(t, CS_ROWS, s, CS_ROWS - 2, 3)  # rows 95,96
                    edge_mask(t, CS_ROWS, 1 - s, CS_ROWS - 3, 3)
                pad_zero(t, CS_ROWS)

            # ---------------- conv3 -> Q/K ----------------
            qt = qkp.tile([128, R * 192 + 64], F16, tag="qt")
            kt = qkp.tile([128, R * 192 + 64], F16, tag="kt")
            for pair, dst in ((0, qt), (1, kt)):
                for y0 in range(r0, r0 + R, 2):
                    ps = pcv.tile([128, 388], F32, tag="cv")
                    for k in range(6):
                        for s in range(2):
                            srct = cst[2 * pair + s]
                            po = s * 64
                            if k < 3:
                                dx = k
                                ca = 1 + (y0 - 1 - (r0 - 1)) * WP + dx - 1
                                nc.tensor.matmul(
                                    ps[po:po + 64, 0:n * WP],
                                    wb[:, C3A[s] + dx * 64:C3A[s] + dx * 64 + 64],
                                    srct[:, ca:ca + n * WP], start=(k == 0),
                                    stop=False, tile_position=(0, po))
                            else:
                                dx = k - 3
                                chb = 1 + (y0 - (r0 - 1)) * WP + dx - 1
                                nc.tensor.matmul(
                                    ps[po:po + 64, 0:n * WP],
                                    wb[:, C3B[s] + dx * 64:C3B[s] + dx * 64 + 64],
                                    srct[:, chb:chb + n * WP], start=False,
                                    stop=(k == 5), tile_position=(0, po))
                    u = y0 - r0
                    copy_bias(
                        dst[:, u * 192:(u + n) * 192].rearrange(
                            "p (r c) -> p r c", c=192),
                        _rr(ps, n, 0)[:, :, 1:193],
                        cb, relu=False)

            # ---------------- attention over R rows ----------------
            # Software-pipelined 3 stages deep: scores(i) | pAB(i-1) | pRL(i-2)
            # so the tensor stream never waits on the ACT/DVE/GpSimd chain.
            # se layout [128, 512]: cols 0:128 v0:128 (w-lo), 128:256 v128:256
            # (M-pad, junk cols 192:256), 256:384 v0:128 (w-hi, K-rows 64:128
            # finite junk killed by zeroed rhs rows), 384:512 v128:256 (w-hi).
            # All 16 apply matmuls are uniform (K=128, M=128, N=64).
            lra = lrp.tile([128, R * 128], F32, tag="lra")
            lrb = lrp.tile([64, R * 128], F32, tag="lrb")

            def apply_mm(p, cols, seT, r_a, r_b):
                o0 = p[:, cols[0]:cols[0] + 64]
                o1 = p[:, cols[1]:cols[1] + 64]
                nc.tensor.matmul(o0, seT[:, 0:128], r_a,
                                 start=True, stop=False)
                nc.tensor.matmul(o0, seT[:, 256:384], r_b,
                                 start=False, stop=True)
                nc.tensor.matmul(o1, seT[:, 128:256], r_a,
                                 start=True, stop=False)
                nc.tensor.matmul(o1, seT[:, 384:512], r_b,
                                 start=False, stop=True)

            def scores_stage(hl):
                qb = hl * 192
                # pb-alternating order: S1, S2, S1T, S2T. Groups share paired
                # psum tiles [128,1024] (2 banks): group g at cols 512g, o0 at
                # +0:192, o1 at +256:448 -> uniform (g,h,c) AP for exp/reduce.
                specs = ((qt, kt, 0), (qt, kt, 64), (kt, qt, 0), (kt, qt, 64))
                se01 = sep.tile([128, 1024], BF16, tag="se01", name="se01")
                se23 = sep.tile([128, 1024], BF16, tag="se23", name="se23")
                z = smp.tile([128, 4], F32, tag="z", name="z")
                for pi, seT in ((0, se01), (1, se23)):
                    p = psc.tile([128, 1024], F32, tag="sc", name="sc")
                    for gi in range(2):
                        i = 2 * pi + gi
                        LS, RS, pb = specs[i]
                        base = 512 * gi
                        rhs = RS[pb:pb + 64, qb:qb + 192]
                        tp = (64, 0) if pb else None
                        nc.tensor.matmul(
                            p[:, base:base + 192],
                            LS[pb:pb + 64, qb:qb + 128],
                            rhs, start=True, stop=True, tile_position=tp)
                        if hl < R - 1:
                            nc.tensor.matmul(
                                p[:, base + 256:base + 448],
                                LS[pb:pb + 64, qb + 128:qb + 256],
                                rhs, start=True, stop=True, tile_position=tp)
                        else:
                            nc.tensor.matmul(
                                p[0:64, base + 256:base + 448],
                                LS[pb:pb + 64, qb + 128:qb + 192],
                                rhs, start=True, stop=True, tile_position=tp)
                    nc.scalar.activation(
                        seT[:, 0:1024].rearrange(
                            "p (g h c) -> p g h c", g=2, c=256)[:, :, :, 0:192],
                        p[:, 0:1024].rearrange(
                            "p (g h c) -> p g h c", g=2, c=256)[:, :, :, 0:192],
                        AF.Exp)
                # softmax row-sums on DVE; z = (z1lo, z1hi, z2lo, z2hi)
                nc.vector.tensor_reduce(
                    z[:, 0:4].rearrange("p (g h) -> p g h", h=2),
                    se01[:, 0:1024].rearrange(
                        "p (g h c) -> p g h c", g=2, c=256)[:, :, :, 0:192],
                    mybir.AxisListType.X, ALU.add)
                iz = smp.tile([128, 4], F32, tag="iz", name="iz")
                nc.vector.reciprocal(iz[:], z[:])
                f1sa = smp.tile([128, 64], BF16, tag="f1sa", name="f1sa")
                f1sb = smp.tile([128, 64], BF16, tag="f1sb", name="f1sb")
                nc.scalar.activation(f1sa[:], fta[:, hl * 128:hl * 128 + 64],
                                     AF.Identity, scale=iz[:, 0:1])
                nc.vector.tensor_scalar(
                    f1sb[:], ftb[:, hl * 128:hl * 128 + 64], iz[:, 1:2],
                    None, ALU.mult)
                return {"se": (se01[:, 0:512], se01[:, 512:1024],
                               se23[:, 0:512], se23[:, 512:1024]),
                        "iz": iz, "f1sa": f1sa, "f1sb": f1sb}

            def pab_stage(hl, st, pAB):
                se, iz = st["se"], st["iz"]
                apply_mm(pAB, (0, 64), se[0], st["f1sa"][:], st["f1sb"][:])
                apply_mm(pAB, (128, 192), se[2],
                         fta[:, hl * 128 + 64:hl * 128 + 128],
                         ftb[:, hl * 128 + 64:hl * 128 + 128])
                ff1a = ffa[:, hl * 128:hl * 128 + 64]
                ff1b = ffb[:, hl * 128:hl * 128 + 64]
                ff2a = ffa[:, hl * 128 + 64:hl * 128 + 128]
                ff2b = ffb[:, hl * 128 + 64:hl * 128 + 128]
                g1a = smp.tile([128, 64], F32, tag="g1a", name="g1a")
                g1b = smp.tile([64, 64], F32, tag="g1b", name="g1b")
                g2a = smp.tile([128, 64], BF16, tag="g2a", name="g2a")
                g2b = g2b_ring[hl % 3]
                nc.vector.scalar_tensor_tensor(
                    g1a[:], pAB[:, 128:192], iz[:, 0:1], ff1a,
                    ALU.mult, ALU.add)
                nc.vector.scalar_tensor_tensor(
                    g1b[:], pAB[0:64, 192:256], iz[0:64, 1:2], ff1b,
                    ALU.mult, ALU.add)
                nc.vector.tensor_add(g2a[:], pAB[:, 0:64], ff2a)
                nc.vector.tensor_add(g2b[0:64, :], pAB[0:64, 64:128], ff2b)
                g1s2a = smp.tile([128, 64], BF16, tag="g1s2a", name="g1s2a")
                g1s2b = gsb_ring[hl % 3]
                nc.scalar.activation(g1s2a[:], g1a[:], AF.Identity,
                                     scale=iz[:, 2:3])
                nc.vector.tensor_scalar(g1s2b[0:64, :], g1b[:],
                                        iz[0:64, 3:4], None, ALU.mult)
                st.update(g1a=g1a, g1b=g1b, g2a=g2a, g2b=g2b, g1s2a=g1s2a,
                          g1s2b=g1s2b)

            def prl_stage(hl, st, pRL):
                se, iz = st["se"], st["iz"]
                apply_mm(pRL, (0, 64), se[1], st["g1s2a"][:], st["g1s2b"][:])
                apply_mm(pRL, (128, 192), se[3], st["g2a"][:], st["g2b"][:])
                # left = g1 + lp*iz2 ; right = g2 + rp  (PSUM reads: DVE only)
                nc.vector.scalar_tensor_tensor(
                    lra[:, hl * 128:hl * 128 + 64], pRL[:, 128:192],
                    iz[:, 2:3], st["g1a"][:], ALU.mult, ALU.add)
                nc.vector.scalar_tensor_tensor(
                    lrb[:, hl * 128:hl * 128 + 64], pRL[0:64, 192:256],
                    iz[0:64, 3:4], st["g1b"][:], ALU.mult, ALU.add)
                nc.vector.tensor_add(
                    lra[:, hl * 128 + 64:hl * 128 + 128], pRL[:, 0:64],
                    st["g2a"][:])
                nc.vector.tensor_add(
                    lrb[:, hl * 128 + 64:hl * 128 + 128], pRL[0:64, 64:128],
                    st["g2b"][0:64, :])

            # prepare next block's inputs + conv1 job list
            if blk + 1 < nblk:
                tl_next = emit_dmas(blk + 1)
                c1t_next, njobs, nfix = conv1_jobs(blk + 1, tl_next)
            else:
                tl_next, c1t_next, njobs, nfix = None, None, [], None

            rowst = {}
            ndone = 0
            for step in range(R + 2):
                if step < R:
                    rowst[step] = scores_stage(step)
                if step >= 1:
                    pap = pap_p.tile([128, 512], F32, tag="pap", name="pap")
                    if 1 <= step <= R:
                        pab_stage(step - 1, rowst[step - 1], pap[:, 0:256])
                    if step >= 2:
                        prl_stage(step - 2, rowst.pop(step - 2), pap[:, 256:512])
                # pace next block's conv1 groups into the stall slots,
                # finishing early enough that the fixup clears before conv2
                want = min((step + 1) * len(njobs) // (R - 1), len(njobs))
                while ndone < want:
                    njobs[ndone]()
                    ndone += 1
                if njobs and ndone == len(njobs) and nfix is not None:
                    nfix()
                    nfix = None

            nc.sync.dma_start(outd[0:128, r0 * 128:(r0 + R) * 128], lra[:])
            nc.sync.dma_start(outd[128:192, r0 * 128:(r0 + R) * 128], lrb[:])
            tl_cur, c1t_cur = tl_next, c1t_next

    nc.compile()
    return nc


# ----------------------------------------------------------------------------
# host-side prep


def _pack_weights(fe1_w1, fe1_b1, fe1_w2, fe1_b2, fe2_w1, fe2_b1, fe2_w2,
                  fe2_b2, conv_w, conv_b):
    wpk = np.zeros((128, WCOLS), np.float32)
    w1 = (np.asarray(fe1_w1, np.float32), np.asarray(fe2_w1, np.float32))
    w2 = (np.asarray(fe1_w2, np.float32), np.asarray(fe2_w2, np.float32))
    w3 = np.asarray(conv_w, np.float32)
    for dx in range(3):
        # conv1: lhsT[K=(dy*64+ci), M=(ws*64+co)]; dy2 at rows 64:128 (0:64 zero)
        for ws in range(2):
            for dy in range(2):
                wpk[dy * 64:dy * 64 + 64,
                    C1A + dx * 128 + ws * 64:C1A + dx * 128 + ws * 64 + 64] = \
                    w1[ws][:, :, dy, dx].T
            wpk[64:128, C1B + dx * 128 + ws * 64:C1B + dx * 128 + ws * 64 + 64] = \
                w1[ws][:, :, 2, dx].T
        # conv2: ws0 normal (rows dy0|dy1, dy2@64:128); ws1 swapped.
        # B columns are read with K=128; rows outside dyb stay zero.
        for ws, wmat in enumerate(w2):
            order = (0, 1) if ws == 0 else (1, 0)
            for k, dy in enumerate(order):
                wpk[k * 64:k * 64 + 64,
                    C2A[ws] + dx * 64:C2A[ws] + dx * 64 + 64] = wmat[:, :, dy, dx].T
            dyb = 64 if ws == 0 else 0
            wpk[dyb:dyb + 64, C2B[ws] + dx * 64:C2B[ws] + dx * 64 + 64] = \
                wmat[:, :, 2, dx].T
        # conv3: variant 0 normal, variant 1 swapped (same weights)
        for v in range(2):
            order = (0, 1) if v == 0 else (1, 0)
            for k, dy in enumerate(order):
                wpk[k * 64:k * 64 + 64,
                    C3A[v] + dx * 64:C3A[v] + dx * 64 + 64] = w3[:, :, dy, dx].T
            dyb = 64 if v == 0 else 0
            wpk[dyb:dyb + 64, C3B[v] + dx * 64:C3B[v] + dx * 64 + 64] = \
                w3[:, :, 2, dx].T
    wpk[np.arange(128), IDC + np.arange(128)] = 1.0
    wpk[0:64, BCOL] = fe1_b1
    wpk[64:128, BCOL] = fe2_b1
    wpk[0:64, BCOL + 1] = fe1_b2
    wpk[64:128, BCOL + 1] = fe2_b2
    wpk[0:64, BCOL + 2] = conv_b
    wpk[64:128, BCOL + 2] = conv_b
    return wpk


def _pad_rows(x, lo, hi):
    """rows [lo, hi) of x[64, H, W], zero fill OOB rows, width pad to 194."""
    n = hi - lo
    out = np.zeros((64, n, WP), np.float32)
    clo, chi = max(lo, 0), min(hi, H)
    if chi > clo:
        out[:, clo - lo:chi - lo, 1:193] = x[:, clo:chi, :]
    return out


def _prep_core(low1, low2, b, h0):
    x1 = _pad_rows(low1[b], h0 - 3, h0 + 100).reshape(64, -1).astype(np.float16)
    x2 = _pad_rows(low2[b], h0 - 3, h0 + 100).reshape(64, -1).astype(np.float16)
    xs = np.concatenate([_pad_rows(low1[b], h0 - 1, h0 + HLOC + 1),
                         _pad_rows(low2[b], h0 - 1, h0 + HLOC + 1)],
                        axis=0).reshape(128, -1).astype(np.float16)
    ft = np.concatenate([low1[b][:, h0:h0 + HLOC, :],
                         low2[b][:, h0:h0 + HLOC, :]], axis=0)  # [128, 96, 192]
    ftf = np.ascontiguousarray(ft.transpose(2, 1, 0)).reshape(192, HLOC * 128)
    ft = ftf.astype(BF)
    top = 1.0 if h0 > 0 else 0.0       # rows -2,-1 valid only for h0=96
    bot = 1.0 if h0 == 0 else 0.0      # rows 96,97 valid only for h0=0
    em = np.empty((128, 6, WP), np.float32)
    em[:, 0, :] = top
    em[:, 1, :] = top
    em[:, 2, :] = 1.0
    em[:, 3, :] = 1.0
    em[:, 4, :] = bot
    em[:, 5, :] = bot
    return {"x1pad": x1, "x2pad": x2, "xskip": xs, "fT": ft, "fF": ftf,
            "emask": em.reshape(128, -1).astype(np.float16)}


_cached = {}


def _get_program(nblk=NBLK):
    if nblk not in _cached:
        _cached[nblk] = build_program(nblk)
    return _cached[nblk]


def run(inputs, trace=False):
    from concourse.bass_utils import run_bass_kernel_spmd

    wpk = _pack_weights(
        inputs["fe1_w1"], inputs["fe1_b1"], inputs["fe1_w2"], inputs["fe1_b2"],
        inputs["fe2_w1"], inputs["fe2_b1"], inputs["fe2_w2"], inputs["fe2_b2"],
        inputs["conv_w"], inputs["conv_b"])
    low1 = np.asarray(inputs["low1"], np.float32)
    low2 = np.asarray(inputs["low2"], np.float32)
    in_maps = []
    for core in range(NCORES):
        b, h0 = core // 2, (core % 2) * HLOC
        m = _prep_core(low1, low2, b, h0)
        m["wpk"] = wpk
        in_maps.append(m)

    nc = _get_program()
    res = run_bass_kernel_spmd(nc, in_maps, list(range(NCORES)), trace=trace)

    left = np.empty((B, C, H, W), np.float32)
    right = np.empty((B, C, H, W), np.float32)
    for core in range(NCORES):
        b, h0 = core // 2, (core % 2) * HLOC
        o = res.results[core]["outT"].reshape(192, HLOC, 128)  # [w, hc, c2]
        left[b, :, h0:h0 + HLOC, :] = o[:, :, 0:64].transpose(2, 1, 0)
        right[b, :, h0:h0 + HLOC, :] = o[:, :, 64:128].transpose(2, 1, 0)
    return (left, right), res


def kernel(**inputs):
    (left, right), _ = run(inputs)
    return (left, right)


# revision 27
# speedup vs baseline: 1.0524x; 1.0524x over previous
"""Trainium2 Bass kernel for the dual-stream conv + cross-width attention module.

Sharding: 8 cores = (batch b, H-half). Core computes rows [h0, h0+96) of batch
b for both outputs (left/right packed along channels).

All conv matmuls are uniform (K=128, M=128, tile (0,0), N=388, fp16): mixing
tile_position / K=64 matmuls into the stream serializes every LDWEIGHTS
against the in-flight matmul and keeps the PE clock cold (measured 460ns/MM
vs 166ns/MM for a uniform stream). dy2 conv1 weights are zero-padded to
K=128; conv2/conv3 hold both fe streams in one tile (partitions = (s, ci))
with block-diagonal weights, and dy is handled by row-offset reads, so no
shifted copies are needed.

On-chip pipeline per 12-row block:
  conv1 (dy01-packed + zero-padded dy2, M=128 = fe1|fe2) -> relu -> c1c tiles
  conv2 (9 MMs: 3dy x 3dx block-diag) + skip-add         -> cs tiles
  conv3 (9 MMs)                                          -> Q1Q2 / K1K2 tiles
  per-row attention: 4 score matmuls (fp32r, N=256-padded), exp+rowsum on ACT,
  4 fused apply matmuls in transposed (w, c) layout, softmax normalization
  folded into per-partition scales. Host untransposes outputs.
"""

import sys

sys.path.insert(0, "/opt/trn_rl_repo")

from contextlib import ExitStack

import numpy as np
import ml_dtypes

import bass_rust
import concourse.bass as bass
import concourse.bacc as bacc
import concourse.mybir as mybir
from concourse import tile
from concourse.vector_clock import ScopedClock
from concourse import tile_utils

# ----------------------------------------------------------------------------
# Workaround: walrus in this container rejects Drain instructions with >1 sem
# wait ("Too many sync wait commands"). Split the TileContext tail drain into
# one-wait-per-Drain.
def _patched_drain_and_barrier(self, tick_clock, wait_clock):
    nc = self.nc
    drain_inst = nc.sync.drain()
    wait_clock.add_sem_waits(
        drain_inst.ins, ScopedClock({None: tick_clock.global_clock})
    )
    si = drain_inst.ins.sync_info
    if si is not None and len(si.on_wait) > 1:
        waits = list(si.on_wait)
        drain_inst.ins.sync_info = bass_rust.SyncInfo(
            on_wait=[waits[0]], on_update=list(si.on_update)
        )
        for w in waits[1:]:
            d = nc.sync.drain()
            d.ins.sync_info = bass_rust.SyncInfo(on_wait=[w], on_update=[])
    nc.all_engine_barrier()
    assert self.sems is not None
    popped = nc._tile_sem_poison_stack.pop()
    assert popped is self._sem_poison
    nc.clear_and_free_semaphores(list(self.sems.allocated().values()))
    nc.all_engine_barrier()


tile.TileContext._drain_and_barrier = _patched_drain_and_barrier
tile_utils.max_sbuf_usage = 206 * 1024

# ----------------------------------------------------------------------------
B, C, H, W = 4, 64, 192, 192
NCORES = 8
HLOC = 96            # rows per core
R = 12               # rows per block
NBLK = HLOC // R
WP = 194             # padded width
F32 = mybir.dt.float32
F32R = mybir.dt.float32r
F16 = mybir.dt.float16
BF16 = mybir.dt.bfloat16
AF = mybir.ActivationFunctionType
ALU = mybir.AluOpType
BF = ml_dtypes.bfloat16

# packed weight column layout (see _pack_weights)
C1A = 0                  # 3 x 128  conv1 dy01 (rows: dy0 ci | dy1 ci; M: fe1|fe2)
C1B = 384                # 3 x 128  conv1 dy2 (rows 0:64 zero, 64:128 dy2 ci)
C2A = [768, 768 + 384]   # per ws: 3 x 64 A (ws1 swapped: dy1|dy0)
C2B = [960, 960 + 384]   # per ws: 3 x 64 B dy2 (K=128 read; zero off-half)
C3A = [1536, 1536 + 384]  # conv3, normal + swapped variants
C3B = [1728, 1728 + 384]
IDC = 2304               # 128-col identity (skip-add via PE accumulation)
BCOL = 2432              # 2432 b1pack, 2433 b2pack, 2434 cbpack
WCOLS = 2435

IN_ROWS = R + 6          # in_dup lo rows   (base row r0-3)
C1_ROWS = R + 4          # c1 tile rows     (base r0-2)
CS_ROWS = R + 2          # cs tile rows     (base r0-1)
XS_ROWS = R + 2          # xskip rows       (base r0-1)


def _rr(ap, rows, lead=1):
    """view of a padded-row region as [P, rows, WP], starting at col `lead`."""
    return ap[:, lead:lead + rows * WP].rearrange("p (r c) -> p r c", c=WP)


def build_program(nblk=NBLK):
    nc = bacc.Bacc("TRN2", target_bir_lowering=False, debug=False,
                   num_devices=NCORES)

    x1d = nc.dram_tensor("x1pad", [64, 103 * WP], F16, kind="ExternalInput").ap()
    x2d = nc.dram_tensor("x2pad", [64, 103 * WP], F16, kind="ExternalInput").ap()
    xsd = nc.dram_tensor("xskip", [128, (HLOC + 2) * WP], F16, kind="ExternalInput").ap()
    ftd = nc.dram_tensor("fT", [192, HLOC * 128], BF16, kind="ExternalInput").ap()
    ffd = nc.dram_tensor("fF", [192, HLOC * 128], F32, kind="ExternalInput").ap()
    wpd = nc.dram_tensor("wpk", [128, WCOLS], F32, kind="ExternalInput").ap()
    emd = nc.dram_tensor("emask", [128, 6 * WP], F16, kind="ExternalInput").ap()
    outd = nc.dram_tensor("outT", [192, HLOC * 128], F32, kind="ExternalOutput").ap()

    with tile.TileContext(nc) as tc, ExitStack() as ctx:
        P = lambda **kw: ctx.enter_context(tc.tile_pool(**kw))
        wpool = P(name="w", bufs=1)
        iop = P(name="io", bufs=2)
        xskp = P(name="xsk", bufs=2)
        c1p = P(name="c1", bufs=1)
        csp = P(name="cs", bufs=1)
        qkp = P(name="qk", bufs=1)
        ftp = P(name="ft", bufs=2)
        lrp = P(name="lr", bufs=1)
        sep = P(name="se", bufs=3)
        smp = P(name="sm", bufs=3)
        pcv = P(name="pcv", bufs=2, space="PSUM")
        psc = P(name="psc", bufs=2, space="PSUM")
        pap_p = P(name="pap", bufs=2, space="PSUM")

        gsb_ring = [wpool.tile([128, 64], BF16, tag=f"gsb{i}", name=f"gsb{i}")
                    for i in range(3)]
        g2b_ring = [wpool.tile([128, 64], BF16, tag=f"g2b{i}", name=f"g2b{i}")
                    for i in range(3)]
        for t in gsb_ring + g2b_ring:
            nc.vector.memset(t[64:128, :], 0.0)
        wf = wpool.tile([128, WCOLS], F32)
        nc.sync.dma_start(wf[:], wpd[:])
        wb = wpool.tile([128, BCOL], F16)
        nc.vector.tensor_copy(wb[:], wf[:, 0:BCOL])
        emk = wpool.tile([128, 6 * WP], F16)
        nc.sync.dma_start(emk[:], emd[:])

        b1 = wf[:, BCOL:BCOL + 1]
        b2 = wf[:, BCOL + 1:BCOL + 2]
        cb = wf[:, BCOL + 2:BCOL + 3]

        copy_ctr = [0]

        def copy_bias(dst_ap, src_ap, bias_ap, relu):
            """dst = [relu](src + bias); alternate DVE/ACT for balance."""
            copy_ctr[0] += 1
            if copy_ctr[0] % 2 == 0:
                nc.scalar.activation(dst_ap, src_ap,
                                     AF.Relu if relu else AF.Identity,
                                     bias=bias_ap)
            elif relu:
                nc.vector.tensor_scalar(dst_ap, src_ap, bias_ap, 0.0,
                                        ALU.add, ALU.max)
            else:
                nc.vector.tensor_scalar(dst_ap, src_ap, bias_ap, None, ALU.add)

        def pad_zero(t, rows):
            nc.gpsimd.memset(
                t[:, 0:rows * WP].rearrange("p (r c) -> p r c", c=WP)[:, :, 0:2],
                0.0)
            nc.gpsimd.memset(t[:, rows * WP:rows * WP + 2], 0.0)

        def lo_slice(s):
            # stream parity s: 0 -> lo rows in partitions 0:64 (loHi),
            #                  1 -> lo rows in partitions 64:128 (hiLo)
            return slice(64, 128) if s else slice(0, 64)

        def hi_dma_inc(t, u, s):
            # after lo rows [u, u+2) are written, copy the shifted hi rows
            # they enable: hi[r] = lo[r+1] for r in [max(u-1,0), u+1).
            lo, hi = (slice(0, 64), slice(64, 128)) if s == 0 else \
                     (slice(64, 128), slice(0, 64))
            rlo = max(u - 1, 0)
            nc.sync.dma_start(t[hi, 1 + rlo * WP:1 + u * WP + WP],
                              t[lo, 1 + (rlo + 1) * WP:1 + (u + 2) * WP])

        def edge_mask(t, rows, s, u, mcol):
            # multiply lo rows [u, u+2) by edge mask rows [mcol, mcol+2)
            sl = lo_slice(s)
            nc.gpsimd.tensor_mul(
                _rr(t, rows)[sl, u:u + 2, 1:193],
                _rr(t, rows)[sl, u:u + 2, 1:193],
                emk[sl, mcol * WP:(mcol + 2) * WP].rearrange(
                    "p (r c) -> p r c", c=WP)[:, :, 1:193])

        n = 2

        def emit_dmas(blk):
            r0 = blk * R
            in1 = iop.tile([128, 2 + IN_ROWS * WP], F16, tag="in1")
            in2 = iop.tile([128, 2 + IN_ROWS * WP], F16, tag="in2")
            for t, src in ((in1, x1d), (in2, x2d)):
                nc.sync.dma_start(t[0:64, 1:1 + IN_ROWS * WP],
                                  src[:, r0 * WP:(r0 + IN_ROWS) * WP])
                nc.sync.dma_start(t[64:128, 1:1 + (IN_ROWS - 1) * WP],
                                  t[0:64, 1 + WP:1 + IN_ROWS * WP])
            xs1 = xskp.tile([128, XS_ROWS * WP], F16, tag="xs1")
            xs2 = xskp.tile([128, XS_ROWS * WP], F16, tag="xs2")
            nc.sync.dma_start(xs1[0:64, :], xsd[0:64, r0 * WP:(r0 + XS_ROWS) * WP])
            nc.sync.dma_start(xs1[64:128, :], xsd[0:64, r0 * WP:(r0 + XS_ROWS) * WP])
            nc.sync.dma_start(xs2[0:64, :], xsd[64:128, r0 * WP:(r0 + XS_ROWS) * WP])
            nc.sync.dma_start(xs2[64:128, :], xsd[64:128, r0 * WP:(r0 + XS_ROWS) * WP])
            fta = ftp.tile([128, R * 128], BF16, tag="fta")
            ftb = ftp.tile([128, R * 128], BF16, tag="ftb")
            nc.sync.dma_start(fta[:], ftd[0:128, r0 * 128:(r0 + R) * 128])
            nc.sync.dma_start(ftb[0:64, :], ftd[128:192, r0 * 128:(r0 + R) * 128])
            # rows 64:128 must be zero: they are the K-padding of the w-hi
            # apply matmul chunks (finite lhsT junk x 0 = 0).
            nc.gpsimd.memset(ftb[64:128, :], 0.0)
            ffa = ftp.tile([128, R * 128], F32, tag="ffa")
            ffb = ftp.tile([64, R * 128], F32, tag="ffb")
            nc.sync.dma_start(ffa[:], ffd[0:128, r0 * 128:(r0 + R) * 128])
            nc.sync.dma_start(ffb[:], ffd[128:192, r0 * 128:(r0 + R) * 128])
            return dict(in1=in1, in2=in2, xs1=xs1, xs2=xs2, fta=fta, ftb=ftb,
                        ffa=ffa, ffb=ffb)

        def conv1_jobs(blk, tl):
            # out rows: [r0-2, r0+R+2); per-stream hiLo tiles for dy01
            # packing. Returned as closures so a block's conv1 can interleave
            # into the previous block's attention steps (fills tensor stalls).
            r0 = blk * R
            c1t = [c1p.tile([128, 2 + C1_ROWS * WP], F16, tag=f"c1_{i}",
                            name=f"c1_{i}") for i in range(4)]
            jobs = []
            for ii0 in range(2):
                for y00 in range(r0 - 2, r0 + R + 2, 2):
                    def job(ii=ii0, y0=y00, r0=r0):
                        ind = (tl["in1"], tl["in2"])[ii]
                        ps = pcv.tile([128, 388], F32, tag="cv", name="cv")
                        for dx in range(3):
                            ca = 1 + (y0 - 1 - (r0 - 3)) * WP + dx - 1
                            chb = 1 + (y0 - (r0 - 3)) * WP + dx - 1
                            nc.tensor.matmul(
                                ps[:, 0:n * WP],
                                wb[:, C1A + dx * 128:C1A + dx * 128 + 128],
                                ind[:, ca:ca + n * WP], start=(dx == 0),
                                stop=False)
                            nc.tensor.matmul(
                                ps[:, 0:n * WP],
                                wb[:, C1B + dx * 128:C1B + dx * 128 + 128],
                                ind[:, chb:chb + n * WP], start=False,
                                stop=(dx == 2))
                        u = y0 - (r0 - 2)
                        for s in range(2):
                            sl = lo_slice(s)
                            psl = slice(s * 64, s * 64 + 64)
                            copy_bias(
                                _rr(c1t[2 * ii + s], C1_ROWS)[sl, u:u + n, 1:193],
                                _rr(ps, n, 0)[psl, :, 1:193],
                                b1[psl], relu=True)
                            hi_dma_inc(c1t[2 * ii + s], u, s)
                    jobs.append(job)

            def fixup():
                for i, t in enumerate(c1t):
                    s = i % 2
                    if blk == 0:
                        edge_mask(t, C1_ROWS, s, 0, 0)            # rows -2,-1
                        # hi[0] = lo[1] (abs -1): em rows (1,2) = (top, 1.0)
                        edge_mask(t, C1_ROWS, 1 - s, 0, 1)
                    if blk == NBLK - 1:
                        edge_mask(t, C1_ROWS, s, C1_ROWS - 2, 4)  # rows 96,97
                        edge_mask(t, C1_ROWS, 1 - s, C1_ROWS - 3, 4)
                    pad_zero(t, C1_ROWS)
            return c1t, jobs, fixup

        tl_cur = emit_dmas(0)
        c1t_cur, jobs0, fix0 = conv1_jobs(0, tl_cur)
        for j in jobs0:
            j()
        fix0()

        for blk in range(nblk):
            r0 = blk * R  # core-relative block start row
            c1t = c1t_cur
            xs1, xs2 = tl_cur["xs1"], tl_cur["xs2"]
            fta, ftb = tl_cur["fta"], tl_cur["ftb"]
            ffa, ffb = tl_cur["ffa"], tl_cur["ffb"]

            # ---------------- conv2 + skip ----------------
            # out rows: [r0-1, r0+R+1). Two K=128,M=64 chains per y0,
            # interleaved at col tiles (0,0)/(0,64) -> concurrent execution.
            cst = [csp.tile([128, 2 + CS_ROWS * WP], F16, tag=f"cs_{i}",
                            name=f"cs_{i}") for i in range(4)]
            for pair, xsk in ((0, xs1), (1, xs2)):
                for y0 in range(r0 - 1, r0 + R + 1, 2):
                    ps = pcv.tile([128, 388], F32, tag="cv")
                    for k in range(6):
                        for s in range(2):
                            srct = c1t[2 * pair + s]
                            po = s * 64
                            if k < 3:
                                dx = k
                                ca = 1 + (y0 - 1 - (r0 - 2)) * WP + dx - 1
                                nc.tensor.matmul(
                                    ps[po:po + 64, 0:n * WP],
                                    wb[:, C2A[s] + dx * 64:C2A[s] + dx * 64 + 64],
                                    srct[:, ca:ca + n * WP], start=(k == 0),
                                    stop=False, tile_position=(0, po))
                            else:
                                dx = k - 3
                                chb = 1 + (y0 - (r0 - 2)) * WP + dx - 1
                                nc.tensor.matmul(
                                    ps[po:po + 64, 0:n * WP],
                                    wb[:, C2B[s] + dx * 64:C2B[s] + dx * 64 + 64],
                                    srct[:, chb:chb + n * WP], start=False,
                                    stop=False, tile_position=(0, po))
                    u = y0 - (r0 - 1)
                    # skip-add folded into the accumulation (identity matmul)
                    nc.tensor.matmul(
                        ps[:, 0:n * WP], wb[:, IDC:IDC + 128],
                        xsk[:, u * WP:(u + n) * WP], start=False, stop=True)
                    for s in range(2):
                        sl = lo_slice(s)
                        psl = slice(s * 64, s * 64 + 64)
                        dst = _rr(cst[2 * pair + s], CS_ROWS)[sl, u:u + n, 1:193]
                        srcp = _rr(ps, n, 0)[psl, :, 1:193]
                        if s == 0:
                            nc.scalar.activation(dst, srcp, AF.Identity,
                                                 bias=b2[psl])
                        else:
                            nc.vector.tensor_scalar(dst, srcp, b2[psl],
                                                    None, ALU.add)
                        hi_dma_inc(cst[2 * pair + s], u, s)

            for i, t in enumerate(cst):
                s = i % 2
                if blk == 0:
                    edge_mask(t, CS_ROWS, s, 0, 1)            # rows -1,0
                    # hi[0] = lo[1] = abs row 0: unmasked; nothing to do
                if blk == NBLK - 1:
                    edge_mask(t, CS_ROWS, s, CS_ROWS - 2, 3)  # rows 95,96
                    edge_mask(t, CS_ROWS, 1 - s, CS_ROWS - 3, 3)
                pad_zero(t, CS_ROWS)

            # ---------------- conv3 -> Q/K ----------------
            qt = qkp.tile([128, R * 192 + 64], F16, tag="qt")
            kt = qkp.tile([128, R * 192 + 64], F16, tag="kt")
            for pair, dst in ((0, qt), (1, kt)):
                for y0 in range(r0, r0 + R, 2):
                    ps = pcv.tile([128, 388], F32, tag="cv")
                    for k in range(6):
                        for s in range(2):
                            srct = cst[2 * pair + s]
                            po = s * 64
                            if k < 3:
                                dx = k
                                ca = 1 + (y0 - 1 - (r0 - 1)) * WP + dx - 1
                                nc.tensor.matmul(
                                    ps[po:po + 64, 0:n * WP],
                                    wb[:, C3A[s] + dx * 64:C3A[s] + dx * 64 + 64],
                                    srct[:, ca:ca + n * WP], start=(k == 0),
                                    stop=False, tile_position=(0, po))
                            else:
                                dx = k - 3
                                chb = 1 + (y0 - (r0 - 1)) * WP + dx - 1
                                nc.tensor.matmul(
                                    ps[po:po + 64, 0:n * WP],
                                    wb[:, C3B[s] + dx * 64:C3B[s] + dx * 64 + 64],
                                    srct[:, chb:chb + n * WP], start=False,
                                    stop=(k == 5), tile_position=(0, po))
                    u = y0 - r0
                    copy_bias(
                        dst[:, u * 192:(u + n) * 192].rearrange(
                            "p (r c) -> p r c", c=192),
                        _rr(ps, n, 0)[:, :, 1:193],
                        cb, relu=False)

            # ---------------- attention over R rows ----------------
            # Software-pipelined 3 stages deep: scores(i) | pAB(i-1) | pRL(i-2)
            # so the tensor stream never waits on the ACT/DVE/GpSimd chain.
            # se layout [128, 512]: cols 0:128 v0:128 (w-lo), 128:256 v128:256
            # (M-pad, junk cols 192:256), 256:384 v0:128 (w-hi, K-rows 64:128
            # finite junk killed by zeroed rhs rows), 384:512 v128:256 (w-hi).
            # All 16 apply matmuls are uniform (K=128, M=128, N=64).
            lra = lrp.tile([128, R * 128], F32, tag="lra")
            lrb = lrp.tile([64, R * 128], F32, tag="lrb")

            def apply_mm(p, cols, seT, r_a, r_b):
                o0 = p[:, cols[0]:cols[0] + 64]
                o1 = p[:, cols[1]:cols[1] + 64]
                nc.tensor.matmul(o0, seT[:, 0:128], r_a,
                                 start=True, stop=False)
                nc.tensor.matmul(o0, seT[:, 256:384], r_b,
                                 start=False, stop=True)
                nc.tensor.matmul(o1, seT[:, 128:256], r_a,
                                 start=True, stop=False)
                nc.tensor.matmul(o1, seT[:, 384:512], r_b,
                                 start=False, stop=True)

            def scores_stage(hl):
                qb = hl * 192
                # pb-alternating order: S1, S2, S1T, S2T. Groups share paired
                # psum tiles [128,1024] (2 banks): group g at cols 512g, o0 at
                # +0:192, o1 at +256:448 -> uniform (g,h,c) AP for exp/reduce.
                specs = ((qt, kt, 0), (qt, kt, 64), (kt, qt, 0), (kt, qt, 64))
                se01 = sep.tile([128, 1024], BF16, tag="se01", name="se01")
                se23 = sep.tile([128, 1024], BF16, tag="se23", name="se23")
                z = smp.tile([128, 4], F32, tag="z", name="z")
                for pi, seT in ((0, se01), (1, se23)):
                    p = psc.tile([128, 1024], F32, tag="sc", name="sc")
                    for gi in range(2):
                        i = 2 * pi + gi
                        LS, RS, pb = specs[i]
                        base = 512 * gi
                        rhs = RS[pb:pb + 64, qb:qb + 192]
                        tp = (64, 0) if pb else None
                        nc.tensor.matmul(
                            p[:, base:base + 192],
                            LS[pb:pb + 64, qb:qb + 128],
                            rhs, start=True, stop=True, tile_position=tp)
                        if hl < R - 1:
                            nc.tensor.matmul(
                                p[:, base + 256:base + 448],
                                LS[pb:pb + 64, qb + 128:qb + 256],
                                rhs, start=True, stop=True, tile_position=tp)
                        else:
                            nc.tensor.matmul(
                                p[0:64, base + 256:base + 448],
                                LS[pb:pb + 64, qb + 128:qb + 192],
                                rhs, start=True, stop=True, tile_position=tp)
                    nc.scalar.activation(
                        seT[:, 0:1024].rearrange(
                            "p (g h c) -> p g h c", g=2, c=256)[:, :, :, 0:192],
                        p[:, 0:1024].rearrange(
                            "p (g h c) -> p g h c", g=2, c=256)[:, :, :, 0:192],
                        AF.Exp)
                # softmax row-sums on DVE; z = (z1lo, z1hi, z2lo, z2hi)
                nc.vector.tensor_reduce(
                    z[:, 0:4].rearrange("p (g h) -> p g h", h=2),
                    se01[:, 0:1024].rearrange(
                        "p (g h c) -> p g h c", g=2, c=256)[:, :, :, 0:192],
                    mybir.AxisListType.X, ALU.add)
                iz = smp.tile([128, 4], F32, tag="iz", name="iz")
                nc.vector.reciprocal(iz[:], z[:])
                f1sa = smp.tile([128, 64], BF16, tag="f1sa", name="f1sa")
                f1sb = smp.tile([128, 64], BF16, tag="f1sb", name="f1sb")
                nc.scalar.activation(f1sa[:], fta[:, hl * 128:hl * 128 + 64],
                                     AF.Identity, scale=iz[:, 0:1])
                nc.vector.tensor_scalar(
                    f1sb[:], ftb[:, hl * 128:hl * 128 + 64], iz[:, 1:2],
                    None, ALU.mult)
                return {"se": (se01[:, 0:512], se01[:, 512:1024],
                               se23[:, 0:512], se23[:, 512:1024]),
                        "iz": iz, "f1sa": f1sa, "f1sb": f1sb}

            def pab_stage(hl, st, pAB):
                se, iz = st["se"], st["iz"]
                apply_mm(pAB, (0, 64), se[0], st["f1sa"][:], st["f1sb"][:])
                apply_mm(pAB, (128, 192), se[2],
                         fta[:, hl * 128 + 64:hl * 128 + 128],
                         ftb[:, hl * 128 + 64:hl * 128 + 128])
                ff1a = ffa[:, hl * 128:hl * 128 + 64]
                ff1b = ffb[:, hl * 128:hl * 128 + 64]
                ff2a = ffa[:, hl * 128 + 64:hl * 128 + 128]
                ff2b = ffb[:, hl * 128 + 64:hl * 128 + 128]
                g1a = smp.tile([128, 64], F32, tag="g1a", name="g1a")
                g1b = smp.tile([64, 64], F32, tag="g1b", name="g1b")
                g2a = smp.tile([128, 64], BF16, tag="g2a", name="g2a")
                g2b = g2b_ring[hl % 3]
                nc.vector.scalar_tensor_tensor(
                    g1a[:], pAB[:, 128:192], iz[:, 0:1], ff1a,
                    ALU.mult, ALU.add)
                nc.vector.scalar_tensor_tensor(
                    g1b[:], pAB[0:64, 192:256], iz[0:64, 1:2], ff1b,
                    ALU.mult, ALU.add)
                nc.vector.tensor_add(g2a[:], pAB[:, 0:64], ff2a)
                nc.vector.tensor_add(g2b[0:64, :], pAB[0:64, 64:128], ff2b)
                g1s2a = smp.tile([128, 64], BF16, tag="g1s2a", name="g1s2a")
                g1s2b = gsb_ring[hl % 3]
                nc.scalar.activation(g1s2a[:], g1a[:], AF.Identity,
                                     scale=iz[:, 2:3])
                nc.vector.tensor_scalar(g1s2b[0:64, :], g1b[:],
                                        iz[0:64, 3:4], None, ALU.mult)
                st.update(g1a=g1a, g1b=g1b, g2a=g2a, g2b=g2b, g1s2a=g1s2a,
                          g1s2b=g1s2b)

            def prl_stage(hl, st, pRL):
                se, iz = st["se"], st["iz"]
                apply_mm(pRL, (0, 64), se[1], st["g1s2a"][:], st["g1s2b"][:])
                apply_mm(pRL, (128, 192), se[3], st["g2a"][:], st["g2b"][:])
                # left = g1 + lp*iz2 ; right = g2 + rp  (PSUM reads: DVE only)
                nc.vector.scalar_tensor_tensor(
                    lra[:, hl * 128:hl * 128 + 64], pRL[:, 128:192],
                    iz[:, 2:3], st["g1a"][:], ALU.mult, ALU.add)
                nc.vector.scalar_tensor_tensor(
                    lrb[:, hl * 128:hl * 128 + 64], pRL[0:64, 192:256],
                    iz[0:64, 3:4], st["g1b"][:], ALU.mult, ALU.add)
                nc.vector.tensor_add(
                    lra[:, hl * 128 + 64:hl * 128 + 128], pRL[:, 0:64],
                    st["g2a"][:])
                nc.vector.tensor_add(
                    lrb[:, hl * 128 + 64:hl * 128 + 128], pRL[0:64, 64:128],
                    st["g2b"][0:64, :])

            # prepare next block's inputs + conv1 job list
            if blk + 1 < nblk:
                tl_next = emit_dmas(blk + 1)
                c1t_next, njobs, nfix = conv1_jobs(blk + 1, tl_next)
            else:
                tl_next, c1t_next, njobs, nfix = None, None, [], None

            rowst = {}
            ndone = 0
            for step in range(R + 2):
                if step < R:
                    rowst[step] = scores_stage(step)
                if step >= 1:
                    pap = pap_p.tile([128, 512], F32, tag="pap", name="pap")
                    if 1 <= step <= R:
                        pab_stage(step - 1, rowst[step - 1], pap[:, 0:256])
                    if step >= 2:
                        prl_stage(step - 2, rowst.pop(step - 2), pap[:, 256:512])
                # pace next block's conv1 groups into the stall slots,
                # finishing early enough that the fixup clears before conv2
                want = min((step + 1) * len(njobs) // (R - 1), len(njobs))
                while ndone < want:
                    njobs[ndone]()
                    ndone += 1
                if njobs and ndone == len(njobs) and nfix is not None:
                    nfix()
                    nfix = None

            nc.sync.dma_start(outd[0:128, r0 * 128:(r0 + R) * 128], lra[:])
            nc.sync.dma_start(outd[128:192, r0 * 128:(r0 + R) * 128], lrb[:])
            tl_cur, c1t_cur = tl_next, c1t_next

    nc.compile()
    return nc


# ----------------------------------------------------------------------------
# host-side prep


def _pack_weights(fe1_w1, fe1_b1, fe1_w2, fe1_b2, fe2_w1, fe2_b1, fe2_w2,
                  fe2_b2, conv_w, conv_b):
    wpk = np.zeros((128, WCOLS), np.float32)
    w1 = (np.asarray(fe1_w1, np.float32), np.asarray(fe2_w1, np.float32))
    w2 = (np.asarray(fe1_w2, np.float32), np.asarray(fe2_w2, np.float32))
    w3 = np.asarray(conv_w, np.float32)
    for dx in range(3):
        # conv1: lhsT[K=(dy*64+ci), M=(ws*64+co)]; dy2 at rows 64:128 (0:64 zero)
        for ws in range(2):
            for dy in range(2):
                wpk[dy * 64:dy * 64 + 64,
                    C1A + dx * 128 + ws * 64:C1A + dx * 128 + ws * 64 + 64] = \
                    w1[ws][:, :, dy, dx].T
            wpk[64:128, C1B + dx * 128 + ws * 64:C1B + dx * 128 + ws * 64 + 64] = \
                w1[ws][:, :, 2, dx].T
        # conv2: ws0 normal (rows dy0|dy1, dy2@64:128); ws1 swapped.
        # B columns are read with K=128; rows outside dyb stay zero.
        for ws, wmat in enumerate(w2):
            order = (0, 1) if ws == 0 else (1, 0)
            for k, dy in enumerate(order):
                wpk[k * 64:k * 64 + 64,
                    C2A[ws] + dx * 64:C2A[ws] + dx * 64 + 64] = wmat[:, :, dy, dx].T
            dyb = 64 if ws == 0 else 0
            wpk[dyb:dyb + 64, C2B[ws] + dx * 64:C2B[ws] + dx * 64 + 64] = \
                wmat[:, :, 2, dx].T
        # conv3: variant 0 normal, variant 1 swapped (same weights)
        for v in range(2):
            order = (0, 1) if v == 0 else (1, 0)
            for k, dy in enumerate(order):
                wpk[k * 64:k * 64 + 64,
                    C3A[v] + dx * 64:C3A[v] + dx * 64 + 64] = w3[:, :, dy, dx].T
            dyb = 64 if v == 0 else 0
            wpk[dyb:dyb + 64, C3B[v] + dx * 64:C3B[v] + dx * 64 + 64] = \
                w3[:, :, 2, dx].T
    wpk[np.arange(128), IDC + np.arange(128)] = 1.0
    wpk[0:64, BCOL] = fe1_b1
    wpk[64:128, BCOL] = fe2_b1
    wpk[0:64, BCOL + 1] = fe1_b2
    wpk[64:128, BCOL + 1] = fe2_b2
    wpk[0:64, BCOL + 2] = conv_b
    wpk[64:128, BCOL + 2] = conv_b
    return wpk


def _pad_rows(x, lo, hi):
    """rows [lo, hi) of x[64, H, W], zero fill OOB rows, width pad to 194."""
    n = hi - lo
    out = np.zeros((64, n, WP), np.float32)
    clo, chi = max(lo, 0), min(hi, H)
    if chi > clo:
        out[:, clo - lo:chi - lo, 1:193] = x[:, clo:chi, :]
    return out


def _prep_core(low1, low2, b, h0):
    x1 = _pad_rows(low1[b], h0 - 3, h0 + 100).reshape(64, -1).astype(np.float16)
    x2 = _pad_rows(low2[b], h0 - 3, h0 + 100).reshape(64, -1).astype(np.float16)
    xs = np.concatenate([_pad_rows(low1[b], h0 - 1, h0 + HLOC + 1),
                         _pad_rows(low2[b], h0 - 1, h0 + HLOC + 1)],
                        axis=0).reshape(128, -1).astype(np.float16)
    ft = np.concatenate([low1[b][:, h0:h0 + HLOC, :],
                         low2[b][:, h0:h0 + HLOC, :]], axis=0)  # [128, 96, 192]
    ftf = np.ascontiguousarray(ft.transpose(2, 1, 0)).reshape(192, HLOC * 128)
    ft = ftf.astype(BF)
    top = 1.0 if h0 > 0 else 0.0       # rows -2,-1 valid only for h0=96
    bot = 1.0 if h0 == 0 else 0.0      # rows 96,97 valid only for h0=0
    em = np.empty((128, 6, WP), np.float32)
    em[:, 0, :] = top
    em[:, 1, :] = top
    em[:, 2, :] = 1.0
    em[:, 3, :] = 1.0
    em[:, 4, :] = bot
    em[:, 5, :] = bot
    return {"x1pad": x1, "x2pad": x2, "xskip": xs, "fT": ft, "fF": ftf,
            "emask": em.reshape(128, -1).astype(np.float16)}


_cached = {}


def _get_program(nblk=NBLK):
    if nblk not in _cached:
        _cached[nblk] = build_program(nblk)
    return _cached[nblk]


def run(inputs, trace=False):
    from concourse.bass_utils import run_bass_kernel_spmd

    wpk = _pack_weights(
        inputs["fe1_w1"], inputs["fe1_b1"], inputs["fe1_w2"], inputs["fe1_b2"],
        inputs["fe2_w1"], inputs["fe2_b1"], inputs["fe2_w2"], inputs["fe2_b2"],
        inputs["conv_w"], inputs["conv_b"])
    low1 = np.asarray(inputs["low1"], np.float32)
    low2 = np.asarray(inputs["low2"], np.float32)
    in_maps = []
    for core in range(NCORES):
        b, h0 = core // 2, (core % 2) * HLOC
        m = _prep_core(low1, low2, b, h0)
        m["wpk"] = wpk
        in_maps.append(m)

    nc = _get_program()
    res = run_bass_kernel_spmd(nc, in_maps, list(range(NCORES)), trace=trace)

    left = np.empty((B, C, H, W), np.float32)
    right = np.empty((B, C, H, W), np.float32)
    for core in range(NCORES):
        b, h0 = core // 2, (core % 2) * HLOC
        o = res.results[core]["outT"].reshape(192, HLOC, 128)  # [w, hc, c2]
        left[b, :, h0:h0 + HLOC, :] = o[:, :, 0:64].transpose(2, 1, 0)
        right[b, :, h0:h0 + HLOC, :] = o[:, :, 64:128].transpose(2, 1, 0)
    return (left, right), res


def kernel(**inputs):
    (left, right), _ = run(inputs)
    return (left, right)
